# revision 1
# baseline (speedup 1.0000x reference)
"""Cost-sensitive loss (CE + cost-matrix lookup) on Trainium2, 8-core data-parallel.

Device work (per core, shard of 32768 rows x 1000 classes, fp32):
  - Stream x in [128, 1000] tiles (2 tiles per DMA).
  - DVE: one blockwise max reduce per tile ([128, 25, 40] -> [128, 25]).
  - ACT: exp(x) with accum_out -> per-row sum(exp) (no max-shift needed;
    |x| <= ~6 so exp never overflows fp32).
  - Exact argmax via hierarchy, batched 8 tiles at a time:
      per-tile max = strided reduce over the 8x25 group maxima,
      max_index over the 200 group maxima -> which 40-wide block per tile,
      per-tile indirect-DMA gather of the winning 40-elem block from HBM
      (HW indirect DMA semantics: one offset per partition, contiguous
      payload -- so one gather instruction per tile),
      one batched max_index over the 8 gathered blocks -> position within.
  - Outputs: per-partition partials [128,1] = sum_t log(sumexp) and the
    predicted argmax table preds [128, 256] (int32).

Host work (O(N) index arithmetic + table lookups):
  - x[row, label[row]] extraction, cost_matrix[label, pred] lookup,
    final sums / division by N.
"""

import numpy as np

import concourse.bacc as bacc
import concourse.bass as bass
import concourse.mybir as mybir
import concourse.tile as tile
from concourse import bass_utils

N = 262144
C = 1000
NCORES = 8
NS = N // NCORES          # 32768 rows per core
P = 128
GK = 40                   # candidate block width (elements)
NG = C // GK              # 25 blocks per row
TB = 8                    # tiles per argmax batch (max_index in_max width)
TPD = 2                   # tiles per streaming DMA

F32 = mybir.dt.float32
I32 = mybir.dt.int32
U32 = mybir.dt.uint32

_CACHE: dict = {}


def _body(tc, nc, x, pbase, partials, g_out, pos_out, nt):
    from contextlib import ExitStack

    nb = nt // TB
    ap_x = x.ap()                                               # [nrows*NG, GK]
    x_tiles = ap_x.rearrange("(t p g) k -> p t (g k)", t=nt, p=P, g=NG)
    AX = mybir.AxisListType.X
    ALU = mybir.AluOpType

    with ExitStack() as ctx:
        const = ctx.enter_context(tc.tile_pool(name="const", bufs=1))
        pbase_sb = const.tile([P, 1], I32)
        s_acc = const.tile([P, nt], F32)
        g_acc = const.tile([P, nt], U32)
        pos_acc = const.tile([P, nt * TB], U32)
        esc = const.tile([P, C], F32)

        nc.sync.dma_start(out=pbase_sb[:], in_=pbase.ap())

        work = ctx.enter_context(tc.tile_pool(name="work", bufs=6))
        xp = ctx.enter_context(tc.tile_pool(name="xp", bufs=12))

        def finish_batch(st):
            """Tail of a batch's argmax: deferred several batches so the
            DVE never stalls on the POOL gather chain. Per-tile max_index
            finds the within-block position; host assembles pred."""
            t0, m8, g8, gbufs = st
            for th in range(TB):
                nc.vector.max_index(
                    out=pos_acc[:, (t0 + th) * TB:(t0 + th + 1) * TB],
                    in_max=m8[:], in_values=gbufs[th][:],
                )

        pending = []
        for b in range(nb):
            t0 = b * TB
            gm = work.tile([P, TB * NG], F32, tag="gm")
            xts = []
            for j in range(TB // TPD):
                xt = xp.tile([P, TPD * C], F32, tag="xt")
                nc.sync.dma_start(
                    out=xt[:].rearrange("p (j c) -> p j c", c=C),
                    in_=x_tiles[:, t0 + j * TPD: t0 + (j + 1) * TPD, :],
                )
                xts.append(xt)
            for th in range(TB):
                sl = xts[th // TPD][:, (th % TPD) * C:(th % TPD + 1) * C]
                nc.vector.reduce_max(
                    out=gm[:, th * NG:(th + 1) * NG],
                    in_=sl.rearrange("p (g k) -> p g k", k=GK),
                    axis=AX,
                )
                nc.scalar.activation(
                    out=esc[:],
                    in_=sl,
                    func=mybir.ActivationFunctionType.Exp,
                    accum_out=s_acc[:, t0 + th: t0 + th + 1],
                )
            # Per-tile maxima of this batch of 8 tiles.
            m8 = work.tile([P, TB], F32, tag="m8")
            nc.vector.reduce_max(
                out=m8[:], in_=gm[:].rearrange("p (t g) -> p t g", g=NG), axis=AX
            )
            g8 = work.tile([P, TB], U32, tag="g8")
            nc.vector.max_index(out=g8[:], in_max=m8[:], in_values=gm[:])
            nc.vector.tensor_copy(out=g_acc[:, t0:t0 + TB], in_=g8[:])
            # Gather each tile's winning 40-elem block: one [128,1]-offset
            # indirect DMA per tile (HW: one descriptor per partition).
            gbufs = []
            for th in range(TB):
                t = t0 + th
                goff = work.tile([P, 1], I32, tag=f"goff{th}")
                # block row-index = g8 + 25p + (3200*t - 25*th)
                nc.vector.scalar_tensor_tensor(
                    out=goff[:], in0=g8[:, th:th + 1],
                    scalar=float(NG * P * t - NG * th),
                    in1=pbase_sb[:], op0=ALU.add, op1=ALU.add,
                )
                gbuf = work.tile([P, GK], F32, tag=f"gbuf{th}")
                nc.gpsimd.indirect_dma_start(
                    out=gbuf[:],
                    out_offset=None,
                    in_=ap_x,
                    in_offset=bass.IndirectOffsetOnAxis(ap=goff[:], axis=0),
                )
                gbufs.append(gbuf)
            pending.append((t0, m8, g8, gbufs))
            if len(pending) > 4:
                finish_batch(pending.pop(0))
        for st in pending:
            finish_batch(st)

        # Epilogue: per-partition sum of log(sumexp).
        ls = const.tile([P, nt], F32)
        nc.scalar.activation(
            out=ls[:], in_=s_acc[:], func=mybir.ActivationFunctionType.Ln
        )
        p1 = const.tile([P, 1], F32)
        nc.vector.reduce_sum(out=p1[:], in_=ls[:], axis=AX)
        nc.sync.dma_start(out=partials.ap(), in_=p1[:])
        nc.sync.dma_start(out=g_out.ap(), in_=g_acc[:])
        nc.sync.dma_start(out=pos_out.ap(), in_=pos_acc[:])


def build_module(nt=NS // P):
    nc = bacc.Bacc(
        "TRN2",
        target_bir_lowering=False,
        debug=False,
        enable_asserts=False,
        num_devices=NCORES,
    )
    x = nc.dram_tensor("x", [nt * P * NG, GK], F32, kind="ExternalInput")
    pbase = nc.dram_tensor("pbase", [P, 1], I32, kind="ExternalInput")
    partials = nc.dram_tensor("partials", [P, 1], F32, kind="ExternalOutput")
    g_out = nc.dram_tensor("g_out", [P, nt], U32, kind="ExternalOutput")
    pos_out = nc.dram_tensor("pos_out", [P, nt * TB], U32, kind="ExternalOutput")
    with tile.TileContext(nc) as tc:
        _body(tc, nc, x, pbase, partials, g_out, pos_out, nt)
    nc.compile()
    return nc


def host_inputs(nt=NS // P, ncores=NCORES, x=None):
    """Per-core input maps. x is the full [N, C] fp32 array."""
    ns = nt * P
    pb = (NG * np.arange(P, dtype=np.int64)[:, None]).astype(np.int32)
    in_maps = []
    for cidx in range(ncores):
        in_maps.append({
            "x": x[cidx * ns:(cidx + 1) * ns].reshape(ns * NG, GK),
            "pbase": pb,
        })
    return in_maps


def combine(results, x, lab, cost_matrix, nt=NS // P):
    """Host-side finish: ce = sum(log sumexp) - sum(x[label]); cost lookup."""
    ns = nt * P
    n_total = len(results) * ns
    lse_sum = 0.0
    preds_all = []
    tmod = (np.arange(nt) % TB).astype(np.int64)                  # [nt]
    for r in results:
        lse_sum += np.asarray(r["partials"], dtype=np.float64).sum()
        g = np.asarray(r["g_out"]).astype(np.int64)               # [P, nt]
        pos = np.asarray(r["pos_out"]).astype(np.int64)           # [P, nt*TB]
        # tile t's within-block position sits at column t*TB + (t % TB)
        w = pos[:, np.arange(nt) * TB + tmod]                     # [P, nt]
        pred = GK * (g - NG * tmod[None, :]) + w                  # [P, nt]
        preds_all.append(pred.T.reshape(-1))
    preds = np.concatenate(preds_all)
    preds = np.clip(preds, 0, C - 1)
    xlab_sum = np.take_along_axis(
        x, lab[: len(preds), None].astype(np.int64), axis=1
    )[:, 0].astype(np.float64).sum()
    cost_sum = np.asarray(cost_matrix)[
        lab[: len(preds)].astype(np.int64), preds
    ].astype(np.float64).sum()
    ce = (lse_sum - xlab_sum) / n_total
    cost = cost_sum / n_total
    return np.float32(ce + cost)


def kernel(outputs, labels, cost_matrix):
    if "nc" not in _CACHE:
        _CACHE["nc"] = build_module()
    nc = _CACHE["nc"]
    x = np.ascontiguousarray(np.asarray(outputs), dtype=np.float32)
    lab = np.asarray(labels)
    in_maps = host_inputs(x=x)
    res = bass_utils.run_bass_kernel_spmd(nc, in_maps, core_ids=list(range(NCORES)))
    return combine(res.results, x, lab, cost_matrix)



# revision 2
# speedup vs baseline: 1.1078x; 1.1078x over previous
"""Cost-sensitive loss (CE + cost-matrix lookup) on Trainium2, 8-core data-parallel.

Device work (per core, shard of 32768 rows x 1000 classes, fp32):
  - Stream x in batches of 8 [128, 1000] tiles: one 4 MB HWDGE DMA per
    batch, rows permuted so each partition's line is 32 KB contiguous in
    HBM. Rings alternate sync/scalar so transfers pipeline.
  - ACT: exp(x) per tile with fp32 accum_out -> per-row sum(exp)
    (|x| <= ~6 so no max-shift needed); the activation OUTPUT (normally
    wasted) is written as fp16 -> esc, an order-preserving all-positive
    key for the argmax.
  - DVE: one grouped reduce_max [128, 8, 1000] -> [128, 8] per batch
    (fp16, 2x mode), then per tile one max_index (FIND_INDEX8) of the
    tile max over the tile's 1000 fp16 values -> exact first-occurrence
    argmax. No gathers, no indirect DMA, no SWDGE traffic.
  - Epilogue: ls = ln(s_acc); partials[128,1] = sum_t ls; DMA out
    partials + the pos table.

Host work (O(N) index arithmetic + table lookups):
  - x[row, label[row]] extraction, cost_matrix[label, pred] lookup,
    final sums / division by N.

fp16 rounding of the argmax key only flips preds on near-ties
(rel gap < 2^-11), which perturbs the cost term by ~1e-4 absolute --
three orders below the 2e-2 relative tolerance.
"""

import numpy as np

import concourse.bacc as bacc
import concourse.bass as bass  # noqa: F401  (kept for API parity)
import concourse.mybir as mybir
import concourse.tile as tile
from concourse import bass_utils

N = 262144
C = 1000
NCORES = 8
NS = N // NCORES          # 32768 rows per core
P = 128
TPB = 8                   # tiles per DMA batch
NT = NS // P              # 256 tiles per core
NB = NT // TPB            # 32 batches per core

F32 = mybir.dt.float32
F16 = mybir.dt.float16
U32 = mybir.dt.uint32

_CACHE: dict = {}


def _body(tc, nc, x, partials, pos_out):
    from contextlib import ExitStack

    AX = mybir.AxisListType.X
    EXP = mybir.ActivationFunctionType.Exp
    LN = mybir.ActivationFunctionType.Ln

    # Row layout: row = b*1024 + p*8 + j  (batch, partition, tile-in-batch)
    # -> per partition each batch is 8 consecutive HBM rows = 32 KB contiguous.
    x_b = x.ap().rearrange("(b p j) c -> p b (j c)", b=NB, p=P, j=TPB)

    with ExitStack() as ctx:
        const = ctx.enter_context(tc.tile_pool(name="const", bufs=1))
        s_acc = const.tile([P, NT], F32)
        pos_acc = const.tile([P, NT * 8], U32)

        xp = ctx.enter_context(tc.tile_pool(name="xp", bufs=2))
        ep = ctx.enter_context(tc.tile_pool(name="ep", bufs=3))
        wk = ctx.enter_context(tc.tile_pool(name="wk", bufs=4))

        for b in range(NB):
            xt = xp.tile([P, TPB * C], F32, tag="xt")
            eng = nc.sync if b % 2 == 0 else nc.scalar
            eng.dma_start(out=xt[:], in_=x_b[:, b, :])

            esc = ep.tile([P, TPB * C], F16, tag="esc")
            for j in range(TPB):
                t = b * TPB + j
                nc.scalar.activation(
                    out=esc[:, j * C:(j + 1) * C],
                    in_=xt[:, j * C:(j + 1) * C],
                    func=EXP,
                    accum_out=s_acc[:, t:t + 1],
                )
            em = wk.tile([P, TPB], F16, tag="em")
            nc.vector.reduce_max(
                out=em[:], in_=esc[:].rearrange("p (j c) -> p j c", c=C), axis=AX
            )
            for j in range(TPB):
                t = b * TPB + j
                nc.vector.max_index(
                    out=pos_acc[:, t * 8:(t + 1) * 8],
                    in_max=em[:],
                    in_values=esc[:, j * C:(j + 1) * C],
                )

        # Epilogue: per-partition sum of log(sumexp).
        ls = const.tile([P, NT], F32)
        nc.scalar.activation(out=ls[:], in_=s_acc[:], func=LN)
        p1 = const.tile([P, 1], F32)
        nc.vector.reduce_sum(out=p1[:], in_=ls[:], axis=AX)
        nc.sync.dma_start(out=partials.ap(), in_=p1[:])
        nc.sync.dma_start(out=pos_out.ap(), in_=pos_acc[:])


def build_module():
    nc = bacc.Bacc(
        "TRN2",
        target_bir_lowering=False,
        debug=False,
        enable_asserts=False,
        num_devices=NCORES,
    )
    x = nc.dram_tensor("x", [NS, C], F32, kind="ExternalInput")
    partials = nc.dram_tensor("partials", [P, 1], F32, kind="ExternalOutput")
    pos_out = nc.dram_tensor("pos_out", [P, NT * 8], U32, kind="ExternalOutput")
    with tile.TileContext(nc) as tc:
        _body(tc, nc, x, partials, pos_out)
    nc.compile()
    return nc


def host_inputs(ncores=NCORES, x=None):
    """Per-core input maps. x is the full [N, C] fp32 array."""
    return [
        {"x": x[cidx * NS:(cidx + 1) * NS]} for cidx in range(ncores)
    ]


def combine(results, x, lab, cost_matrix):
    """Host-side finish: ce = sum(log sumexp) - sum(x[label]); cost lookup."""
    n_total = len(results) * NS
    lse_sum = 0.0
    preds_all = []
    j8 = np.arange(TPB)
    for r in results:
        lse_sum += np.asarray(r["partials"], dtype=np.float64).sum()
        pos = np.asarray(r["pos_out"]).astype(np.int64)       # [P, NT*8]
        pos = pos.reshape(P, NB, TPB, 8)
        w = pos[:, :, j8, j8]                                 # [P, NB, TPB]
        # row = b*1024 + p*8 + j
        preds_all.append(np.transpose(w, (1, 0, 2)).reshape(-1))
    preds = np.concatenate(preds_all)
    preds = np.clip(preds, 0, C - 1)
    xlab_sum = np.take_along_axis(
        x, lab[:, None].astype(np.int64), axis=1
    )[:, 0].astype(np.float64).sum()
    cost_sum = np.asarray(cost_matrix)[
        lab.astype(np.int64), preds
    ].astype(np.float64).sum()
    ce = (lse_sum - xlab_sum) / n_total
    cost = cost_sum / n_total
    return np.float32(ce + cost)


def kernel(outputs, labels, cost_matrix):
    if "nc" not in _CACHE:
        _CACHE["nc"] = build_module()
    nc = _CACHE["nc"]
    x = np.ascontiguousarray(np.asarray(outputs), dtype=np.float32)
    lab = np.asarray(labels)
    in_maps = host_inputs(x=x)
    res = bass_utils.run_bass_kernel_spmd(nc, in_maps, core_ids=list(range(NCORES)))
    return combine(res.results, x, lab, cost_matrix)


# revision 4
# speedup vs baseline: 1.5458x; 1.3953x over previous
"""Cost-sensitive loss (CE + cost-matrix lookup) on Trainium2, 8-core data-parallel.

Device work (per core, shard of 32768 rows x 1000 classes, fp32):
  - Stream x in batches of 8 [128, 1000] tiles: one 4 MB HWDGE DMA per
    batch, rows permuted so each partition's line is 32 KB contiguous in
    HBM. Rings alternate sync/scalar so transfers pipeline.
  - ACT: exp(x) per tile with fp32 accum_out -> per-row sum(exp)
    (|x| <= ~6 so no max-shift needed); the activation OUTPUT (normally
    wasted) is written as fp16 -> esc, an order-preserving all-positive
    argmax key (the DVE runs every op at ~1 elem/cycle/lane, so the
    argmax budget is a single full pass over the data).
  - DVE (the one full pass): grouped reduce_max over 25 blocks of 40
    -> gm [128, 25] fp16 per tile. Then two small batched ops per
    8-tile batch: z = gm_bits * 32 + block_id (scalar_tensor_tensor on
    the uint16 view; positive fp16 bit patterns sort like the values),
    and a grouped reduce_max z [128, 8, 25] -> zb [128, 8] uint32.
    max(z) is lexicographic (block max, block id), so zb & 31 is the
    argmax block exactly (fp16 ties resolve to the largest block id).
  - Epilogue: ls = ln(s_acc); partials[128,1] = sum_t ls; DMA out
    partials + zb table.

Host work (O(N) with small constants):
  - decode winning block, exact fp32 argmax within the 40-wide block,
    x[row, label[row]] extraction, cost_matrix[label, pred] lookup,
    final sums / division by N.

fp16 rounding only affects which near-tied BLOCK wins (~0.4% of rows);
within the block the host argmax is exact. The cost-term perturbation
is ~1e-4 absolute, three orders below the 2e-2 relative tolerance.
"""

import numpy as np

import concourse.bacc as bacc
import concourse.bass as bass  # noqa: F401  (kept for API parity)
import concourse.mybir as mybir
import concourse.tile as tile
from concourse import bass_utils

N = 262144
C = 1000
NCORES = 8
NS = N // NCORES          # 32768 rows per core
P = 128
TPB = 8                   # tiles per DMA batch
NT = NS // P              # 256 tiles per core
NB = NT // TPB            # 32 batches per core
G = 25                    # blocks per row
W = C // G                # block width (40)
S = 32                    # block-id stride in the packed key

F32 = mybir.dt.float32
F16 = mybir.dt.float16
U16 = mybir.dt.uint16
U32 = mybir.dt.uint32

_CACHE: dict = {}


def _body(tc, nc, x, blkc, partials, zb_out):
    from contextlib import ExitStack

    AX = mybir.AxisListType.X
    ALU = mybir.AluOpType
    EXP = mybir.ActivationFunctionType.Exp
    LN = mybir.ActivationFunctionType.Ln

    # Row layout: row = b*1024 + p*8 + j  (batch, partition, tile-in-batch)
    # -> per partition each batch is 8 consecutive HBM rows = 32 KB contiguous.
    x_b = x.ap().rearrange("(b p j) c -> p b (j c)", b=NB, p=P, j=TPB)

    with ExitStack() as ctx:
        const = ctx.enter_context(tc.tile_pool(name="const", bufs=1))
        s_acc = const.tile([P, NT], F32)
        zb_acc = const.tile([P, NT], U32)
        blkc_sb = const.tile([P, TPB * G], U32)
        nc.sync.dma_start(out=blkc_sb[:], in_=blkc.ap())

        xp = ctx.enter_context(tc.tile_pool(name="xp", bufs=2))
        ep = ctx.enter_context(tc.tile_pool(name="ep", bufs=3))
        wk = ctx.enter_context(tc.tile_pool(name="wk", bufs=3))

        for b in range(NB):
            xt = xp.tile([P, TPB * C], F32, tag="xt")
            eng = nc.sync if b % 2 == 0 else nc.scalar
            eng.dma_start(out=xt[:], in_=x_b[:, b, :])

            esc = ep.tile([P, TPB * C], F16, tag="esc")
            gm = wk.tile([P, TPB * G], F16, tag="gm")
            z = wk.tile([P, TPB * G], U32, tag="z")
            for j in range(TPB):
                t = b * TPB + j
                nc.scalar.activation(
                    out=esc[:, j * C:(j + 1) * C],
                    in_=xt[:, j * C:(j + 1) * C],
                    func=EXP,
                    accum_out=s_acc[:, t:t + 1],
                )
                nc.vector.reduce_max(
                    out=gm[:, j * G:(j + 1) * G],
                    in_=esc[:, j * C:(j + 1) * C].rearrange(
                        "p (g w) -> p g w", w=W
                    ),
                    axis=AX,
                )
            # z = gm_bits * S + blk ; max(z) = (block max, block id) lexicographic
            nc.vector.scalar_tensor_tensor(
                out=z[:],
                in0=gm[:].bitcast(U16),
                scalar=S,
                in1=blkc_sb[:],
                op0=ALU.mult,
                op1=ALU.add,
            )
            nc.vector.reduce_max(
                out=zb_acc[:, b * TPB:(b + 1) * TPB],
                in_=z[:].rearrange("p (j g) -> p j g", g=G),
                axis=AX,
            )

        # Epilogue: per-partition sum of log(sumexp).
        ls = const.tile([P, NT], F32)
        nc.scalar.activation(out=ls[:], in_=s_acc[:], func=LN)
        p1 = const.tile([P, 1], F32)
        nc.vector.reduce_sum(out=p1[:], in_=ls[:], axis=AX)
        nc.sync.dma_start(out=partials.ap(), in_=p1[:])
        nc.sync.dma_start(out=zb_out.ap(), in_=zb_acc[:])


def build_module():
    nc = bacc.Bacc(
        "TRN2",
        target_bir_lowering=False,
        debug=False,
        enable_asserts=False,
        num_devices=NCORES,
    )
    x = nc.dram_tensor("x", [NS, C], F32, kind="ExternalInput")
    blkc = nc.dram_tensor("blkc", [P, TPB * G], U32, kind="ExternalInput")
    partials = nc.dram_tensor("partials", [P, 1], F32, kind="ExternalOutput")
    zb_out = nc.dram_tensor("zb_out", [P, NT], U32, kind="ExternalOutput")
    with tile.TileContext(nc) as tc:
        _body(tc, nc, x, blkc, partials, zb_out)
    nc.compile()
    return nc


def host_inputs(ncores=NCORES, x=None):
    """Per-core input maps. x is the full [N, C] fp32 array."""
    blkc = np.broadcast_to(
        np.tile(np.arange(G, dtype=np.uint32), TPB), (P, TPB * G)
    ).copy()
    return [
        {"x": x[cidx * NS:(cidx + 1) * NS], "blkc": blkc}
        for cidx in range(ncores)
    ]


def combine(results, x, lab, cost_matrix):
    """Host-side finish: ce = sum(log sumexp) - sum(x[label]); cost lookup."""
    n_total = len(results) * NS
    lse_sum = 0.0
    blk_all = []
    for r in results:
        lse_sum += np.asarray(r["partials"], dtype=np.float64).sum()
        zb = np.asarray(r["zb_out"]).astype(np.int64)         # [P, NT]
        blk = zb & (S - 1)                                    # winning block id
        # col t = b*8 + j; row = b*1024 + p*8 + j
        wv = blk.reshape(P, NB, TPB)
        blk_all.append(np.transpose(wv, (1, 0, 2)).reshape(-1))
    blk = np.clip(np.concatenate(blk_all), 0, G - 1)          # [N]
    # Exact fp32 argmax within the winning 40-wide block.
    base = blk * W
    rows = np.arange(n_total, dtype=np.int64)[:, None]
    inner = np.argmax(x[rows, base[:, None] + np.arange(W)[None, :]], axis=1)
    preds = base + inner
    xlab_sum = np.take_along_axis(
        x, lab[:, None].astype(np.int64), axis=1
    )[:, 0].astype(np.float64).sum()
    cost_sum = np.asarray(cost_matrix)[
        lab.astype(np.int64), preds
    ].astype(np.float64).sum()
    ce = (lse_sum - xlab_sum) / n_total
    cost = cost_sum / n_total
    return np.float32(ce + cost)


def kernel(outputs, labels, cost_matrix):
    if "nc" not in _CACHE:
        _CACHE["nc"] = build_module()
    nc = _CACHE["nc"]
    x = np.ascontiguousarray(np.asarray(outputs), dtype=np.float32)
    lab = np.asarray(labels)
    in_maps = host_inputs(x=x)
    res = bass_utils.run_bass_kernel_spmd(nc, in_maps, core_ids=list(range(NCORES)))
    return combine(res.results, x, lab, cost_matrix)


# revision 5
# speedup vs baseline: 1.5595x; 1.0089x over previous
"""Cost-sensitive loss (CE + cost-matrix lookup) on Trainium2, 8-core data-parallel.

Device work (per core, shard of 32768 rows x 1000 classes, fp32):
  - Stream x in batches of 8 [128, 1000] tiles: one 4 MB HWDGE DMA per
    batch, rows permuted so each partition's line is 32 KB contiguous in
    HBM. Rings alternate sync/scalar so transfers pipeline.
  - ACT: exp(x) per tile with fp32 accum_out -> per-row sum(exp)
    (|x| <= ~6 so no max-shift needed); the activation OUTPUT (normally
    wasted) is written as fp16 -> esc, an order-preserving all-positive
    argmax key (the DVE runs every op at ~1 elem/cycle/lane, so the
    argmax budget is a single full pass over the data).
  - DVE (the one full pass): grouped reduce_max over 25 blocks of 40
    -> gm [128, 25] fp16 per tile. Then two small batched ops per
    8-tile batch: z = gm_bits * 32 + block_id (scalar_tensor_tensor on
    the uint16 view; positive fp16 bit patterns sort like the values),
    and a grouped reduce_max z [128, 8, 25] -> zb [128, 8] uint32.
    max(z) is lexicographic (block max, block id), so zb & 31 is the
    argmax block exactly (fp16 ties resolve to the largest block id).
  - Epilogue: ls = ln(s_acc); partials[128,1] = sum_t ls; DMA out
    partials + zb table.

Host work (O(N) with small constants):
  - decode winning block, exact fp32 argmax within the 40-wide block,
    x[row, label[row]] extraction, cost_matrix[label, pred] lookup,
    final sums / division by N.

fp16 rounding only affects which near-tied BLOCK wins (~0.4% of rows);
within the block the host argmax is exact. The cost-term perturbation
is ~1e-4 absolute, three orders below the 2e-2 relative tolerance.
"""

import numpy as np

import concourse.bacc as bacc
import concourse.bass as bass  # noqa: F401  (kept for API parity)
import concourse.mybir as mybir
import concourse.tile as tile
from concourse import bass_utils

N = 262144
C = 1000
NCORES = 8
NS = N // NCORES          # 32768 rows per core
P = 128
TPB = 8                   # tiles per DMA batch
NT = NS // P              # 256 tiles per core
NB = NT // TPB            # 32 batches per core
G = 25                    # blocks per row
W = C // G                # block width (40)
S = 32                    # block-id stride in the packed key

F32 = mybir.dt.float32
F16 = mybir.dt.float16
U16 = mybir.dt.uint16
U32 = mybir.dt.uint32

_CACHE: dict = {}


def _body(tc, nc, x, blkc, partials, zb_out):
    from contextlib import ExitStack

    AX = mybir.AxisListType.X
    ALU = mybir.AluOpType
    EXP = mybir.ActivationFunctionType.Exp
    LN = mybir.ActivationFunctionType.Ln

    # Row layout: row = b*1024 + p*8 + j  (batch, partition, tile-in-batch)
    # -> per partition each batch is 8 consecutive HBM rows = 32 KB contiguous.
    x_b = x.ap().rearrange("(b p j) c -> p b (j c)", b=NB, p=P, j=TPB)

    with ExitStack() as ctx:
        const = ctx.enter_context(tc.tile_pool(name="const", bufs=1))
        s_acc = const.tile([P, NT], F32)
        zb_acc = const.tile([P, NT], U32)
        blkc_sb = const.tile([P, TPB * G], U32)
        nc.sync.dma_start(out=blkc_sb[:], in_=blkc.ap())

        xp = ctx.enter_context(tc.tile_pool(name="xp", bufs=2))
        ep = ctx.enter_context(tc.tile_pool(name="ep", bufs=3))
        wk = ctx.enter_context(tc.tile_pool(name="wk", bufs=3))

        for bd in range(NB // 2):
            xt = xp.tile([P, 2 * TPB * C], F32, tag="xt")
            eng = nc.sync if bd % 2 == 0 else nc.scalar
            eng.dma_start(
                out=xt[:].rearrange("p (h jc) -> p h jc", h=2),
                in_=x_b[:, 2 * bd:2 * bd + 2, :],
            )
            for h in range(2):
                b = 2 * bd + h
                xh = xt[:, h * TPB * C:(h + 1) * TPB * C]
                esc = ep.tile([P, TPB * C], F16, tag="esc")
                gm = wk.tile([P, TPB * G], F16, tag="gm")
                z = wk.tile([P, TPB * G], U32, tag="z")
                for j in range(TPB):
                    t = b * TPB + j
                    nc.scalar.activation(
                        out=esc[:, j * C:(j + 1) * C],
                        in_=xh[:, j * C:(j + 1) * C],
                        func=EXP,
                        accum_out=s_acc[:, t:t + 1],
                    )
                    nc.vector.reduce_max(
                        out=gm[:, j * G:(j + 1) * G],
                        in_=esc[:, j * C:(j + 1) * C].rearrange(
                            "p (g w) -> p g w", w=W
                        ),
                        axis=AX,
                    )
                # z = gm_bits*S + blk ; max(z) = (block max, block id) lex.
                nc.vector.scalar_tensor_tensor(
                    out=z[:],
                    in0=gm[:].bitcast(U16),
                    scalar=S,
                    in1=blkc_sb[:],
                    op0=ALU.mult,
                    op1=ALU.add,
                )
                nc.vector.reduce_max(
                    out=zb_acc[:, b * TPB:(b + 1) * TPB],
                    in_=z[:].rearrange("p (j g) -> p j g", g=G),
                    axis=AX,
                )

        # Epilogue: per-partition sum of log(sumexp).
        ls = const.tile([P, NT], F32)
        nc.scalar.activation(out=ls[:], in_=s_acc[:], func=LN)
        p1 = const.tile([P, 1], F32)
        nc.vector.reduce_sum(out=p1[:], in_=ls[:], axis=AX)
        nc.sync.dma_start(out=partials.ap(), in_=p1[:])
        nc.sync.dma_start(out=zb_out.ap(), in_=zb_acc[:])


def build_module():
    nc = bacc.Bacc(
        "TRN2",
        target_bir_lowering=False,
        debug=False,
        enable_asserts=False,
        num_devices=NCORES,
    )
    x = nc.dram_tensor("x", [NS, C], F32, kind="ExternalInput")
    blkc = nc.dram_tensor("blkc", [P, TPB * G], U32, kind="ExternalInput")
    partials = nc.dram_tensor("partials", [P, 1], F32, kind="ExternalOutput")
    zb_out = nc.dram_tensor("zb_out", [P, NT], U32, kind="ExternalOutput")
    with tile.TileContext(nc) as tc:
        _body(tc, nc, x, blkc, partials, zb_out)
    nc.compile()
    return nc


def host_inputs(ncores=NCORES, x=None):
    """Per-core input maps. x is the full [N, C] fp32 array."""
    blkc = np.broadcast_to(
        np.tile(np.arange(G, dtype=np.uint32), TPB), (P, TPB * G)
    ).copy()
    return [
        {"x": x[cidx * NS:(cidx + 1) * NS], "blkc": blkc}
        for cidx in range(ncores)
    ]


def combine(results, x, lab, cost_matrix):
    """Host-side finish: ce = sum(log sumexp) - sum(x[label]); cost lookup."""
    n_total = len(results) * NS
    lse_sum = 0.0
    blk_all = []
    for r in results:
        lse_sum += np.asarray(r["partials"], dtype=np.float64).sum()
        zb = np.asarray(r["zb_out"]).astype(np.int64)         # [P, NT]
        blk = zb & (S - 1)                                    # winning block id
        # col t = b*8 + j; row = b*1024 + p*8 + j
        wv = blk.reshape(P, NB, TPB)
        blk_all.append(np.transpose(wv, (1, 0, 2)).reshape(-1))
    blk = np.clip(np.concatenate(blk_all), 0, G - 1)          # [N]
    # Exact fp32 argmax within the winning 40-wide block.
    base = blk * W
    rows = np.arange(n_total, dtype=np.int64)[:, None]
    inner = np.argmax(x[rows, base[:, None] + np.arange(W)[None, :]], axis=1)
    preds = base + inner
    xlab_sum = np.take_along_axis(
        x, lab[:, None].astype(np.int64), axis=1
    )[:, 0].astype(np.float64).sum()
    cost_sum = np.asarray(cost_matrix)[
        lab.astype(np.int64), preds
    ].astype(np.float64).sum()
    ce = (lse_sum - xlab_sum) / n_total
    cost = cost_sum / n_total
    return np.float32(ce + cost)


def kernel(outputs, labels, cost_matrix):
    if "nc" not in _CACHE:
        _CACHE["nc"] = build_module()
    nc = _CACHE["nc"]
    x = np.ascontiguousarray(np.asarray(outputs), dtype=np.float32)
    lab = np.asarray(labels)
    in_maps = host_inputs(x=x)
    res = bass_utils.run_bass_kernel_spmd(nc, in_maps, core_ids=list(range(NCORES)))
    return combine(res.results, x, lab, cost_matrix)
